# revision 11
# baseline (speedup 1.0000x reference)
"""TRN2 Bass kernel for nn_BilinearTensorProduct.

  out = tanh(concat(V1,V2) @ W + einsum('bd,kde,be->bk', V1, T, V2) + sum(b))
  B=8192, D=256, K=64.  Data-parallel: batch sharded 8 ways, T/W/b replicated.

Bilinear path: fp16 hi pass + two fp8-e4m3 DoubleRow correction passes.
With A1 = fp16(V1*2^11), A2 = V1*2^11 - A1, TH = fp16(T*2^11), TL = T*2^11-TH:

  V1*T*2^22 = A1*TH  +  A2*T*2^11  +  A1*TL            (exactly)

Pass 1 runs in fp16 (2 matmuls over the 128-row contraction chunks).  Passes
2 and 3 run as single fp8 DoubleRow matmuls each (256-deep contraction, 0.5
cycles/row): operands are rescaled into e4m3 range with scales chosen so
every pass lands in the same 2^22-scaled PSUM group (X2 = f8(A2*48),
Y2 = f8(T*2^22/(2^11*48)), X3 = f8(A1*48/2^11), Y3 = f8(TL*2^22/(2^11*48))).
The fused DVE affine_mul_reduce then multiplies by fp32 V2 and row-reduces
with scale 2^-22.  Feedforward path and epilogue as before (fp16 3-pass with
sum(b) folded in as a bias contraction block; tanh on the scalar engine).
T streams split across the sync and gpsimd DMA queues; constants ride the
scalar queue.
"""

import numpy as np
import ml_dtypes
from contextlib import ExitStack

import concourse.bacc as bacc
import concourse.mybir as mybir
from concourse import tile
from concourse import bass_utils

B, D, K = 8192, 256, 64
NCORES = 8
BS = B // NCORES          # 1024 batch rows per core
NBT = BS // 128           # 8 b-tiles of 128 rows
NC_FF = 5                 # ff contraction chunks: 4 real + 1 bias block
S = 2048.0                # 2^11 hi/lo split point
SX = 48.0                 # fp8 V1-side scale
SY = 2.0 ** 22 / (S * SX)  # fp8 T-side scale; SX*SY*S = 2^22
UNSCALE = 2.0 ** -22

f32 = mybir.dt.float32
f16 = mybir.dt.float16
f8 = mybir.dt.float8e4
e4m3 = ml_dtypes.float8_e4m3

_NC_CACHE = {}


def _build():
    nc = bacc.Bacc("TRN2", target_bir_lowering=False, debug=False)
    A1 = nc.dram_tensor("A1", [D, BS], f16, kind="ExternalInput")
    X2 = nc.dram_tensor("X2", [128, 2, BS], f8, kind="ExternalInput")
    X3 = nc.dram_tensor("X3", [128, 2, BS], f8, kind="ExternalInput")
    TH = nc.dram_tensor("TH", [K, D, D], f16, kind="ExternalInput")
    Y2 = nc.dram_tensor("Y2", [K, 128, 2, D], f8, kind="ExternalInput")
    Y3 = nc.dram_tensor("Y3", [K, 128, 2, D], f8, kind="ExternalInput")
    V2N = nc.dram_tensor("V2N", [BS, D], f32, kind="ExternalInput")
    CTH = nc.dram_tensor("CTH", [NC_FF * 128, BS], f16, kind="ExternalInput")
    CTL = nc.dram_tensor("CTL", [NC_FF * 128, BS], f16, kind="ExternalInput")
    WH = nc.dram_tensor("WH", [NC_FF * 128, K], f16, kind="ExternalInput")
    WL = nc.dram_tensor("WL", [NC_FF * 128, K], f16, kind="ExternalInput")
    OUT = nc.dram_tensor("OUT", [BS, K], f32, kind="ExternalOutput")

    DR = mybir.MatmulPerfMode.DoubleRow

    with tile.TileContext(nc) as tc:
        with ExitStack() as ctx:
            const = ctx.enter_context(tc.tile_pool(name="const", bufs=1))
            tpool = ctx.enter_context(tc.tile_pool(name="tpool", bufs=6))
            psb = ctx.enter_context(tc.tile_pool(name="psb", bufs=7, space="PSUM"))
            psff = ctx.enter_context(tc.tile_pool(name="psff", bufs=1, space="PSUM"))
            scr = ctx.enter_context(tc.tile_pool(name="scr", bufs=3))

            a1 = [const.tile([128, BS], f16, name=f"a1_{c}", tag=f"a1_{c}")
                  for c in range(2)]
            x2 = const.tile([128, 2, BS], f8, name="x2", tag="x2")
            x3 = const.tile([128, 2, BS], f8, name="x3", tag="x3")
            v2 = [const.tile([128, D], f32, name=f"v2_{t}", tag=f"v2_{t}")
                  for t in range(NBT)]
            cth = [const.tile([128, BS], f16, name=f"cth_{c}", tag=f"cth_{c}")
                   for c in range(NC_FF)]
            ctl = [const.tile([128, BS], f16, name=f"ctl_{c}", tag=f"ctl_{c}")
                   for c in range(NC_FF)]
            wth = [const.tile([128, K], f16, name=f"wth_{c}", tag=f"wth_{c}")
                   for c in range(NC_FF)]
            wtl = [const.tile([128, K], f16, name=f"wtl_{c}", tag=f"wtl_{c}")
                   for c in range(NC_FF)]
            bil = [const.tile([128, K], f32, name=f"bil_{t}", tag=f"bil_{t}")
                   for t in range(NBT)]

            # startup-critical tensors spread across all three queues so the
            # first matmul unit can issue within ~4us
            nc.scalar.dma_start(a1[0][:], A1.ap()[0:128, :])
            nc.scalar.dma_start(x2[:], X2.ap())
            nc.scalar.dma_start(a1[1][:], A1.ap()[128:256, :])
            nc.scalar.dma_start(x3[:], X3.ap())
            for t in range(NBT):
                nc.scalar.dma_start(v2[t][:], V2N.ap()[t * 128:(t + 1) * 128, :])
            for c in range(NC_FF):
                nc.scalar.dma_start(cth[c][:], CTH.ap()[c * 128:(c + 1) * 128, :])
                nc.scalar.dma_start(ctl[c][:], CTL.ap()[c * 128:(c + 1) * 128, :])
                nc.scalar.dma_start(wth[c][:], WH.ap()[c * 128:(c + 1) * 128, :])
                nc.scalar.dma_start(wtl[c][:], WL.ap()[c * 128:(c + 1) * 128, :])

            ffp = psff.tile([128, NBT * K], f32, name="ffp", tag="ffp")

            n_kp = K // 2
            for kp in range(n_kp):
                k = 2 * kp
                th0 = tpool.tile([128, 2 * D], f16, tag="th0")
                th1 = tpool.tile([128, 2 * D], f16, tag="th1")
                y2t = tpool.tile([128, 2, 2 * D], f8, tag="y2t")
                y3t = tpool.tile([128, 2, 2 * D], f8, tag="y3t")
                q0 = nc.sync if kp % 2 == 0 else nc.gpsimd
                q1 = nc.gpsimd if kp % 2 == 0 else nc.sync
                # th tiles first: the fp16 half-groups' semaphore waits then
                # cover only the th DMAs; y tiles are queued after (consumed
                # later by the DR half-groups)
                q0.dma_start(th0[:, 0:D], TH.ap()[k, 0:128, :])
                q0.dma_start(th0[:, D:2 * D], TH.ap()[k + 1, 0:128, :])
                q1.dma_start(th1[:, 0:D], TH.ap()[k, 128:256, :])
                q1.dma_start(th1[:, D:2 * D], TH.ap()[k + 1, 128:256, :])
                q0.dma_start(y2t[:, :, 0:D], Y2.ap()[k])
                q0.dma_start(y2t[:, :, D:2 * D], Y2.ap()[k + 1])
                q1.dma_start(y3t[:, :, 0:D], Y3.ap()[k])
                q1.dma_start(y3t[:, :, D:2 * D], Y3.ap()[k + 1])
                if kp == 12:
                    # feedforward (+sum(b) bias block) as fp16 hi/lo 3-pass;
                    # CT side is pre-scaled 2^11, unscaled in the epilogue
                    for t in range(NBT):
                        passes = [(cth, wth), (cth, wtl), (ctl, wth)]
                        n_mm = len(passes) * NC_FF
                        i_mm = 0
                        for cs, ws in passes:
                            for c in range(NC_FF):
                                nc.tensor.matmul(
                                    ffp[:, t * K:(t + 1) * K],
                                    cs[c][:, t * 128:(t + 1) * 128],
                                    ws[c][:],
                                    start=(i_mm == 0), stop=(i_mm == n_mm - 1),
                                )
                                i_mm += 1
                # half-groups of 4 b-tiles: batch the fp16 matmuls, then the
                # DR matmuls — each normal<->DR mode switch serializes the PE
                # (exposes the 136ns DR LDWEIGHTS), so amortize switches over
                # 8 matmuls instead of paying 2 per tile
                for half in range(2):
                    tg = range(half * 4, half * 4 + 4)
                    pss = {}
                    for t in tg:
                        bsl = slice(t * 128, (t + 1) * 128)
                        ps = pss[t] = psb.tile([128, 2 * D], f32, name="ps", tag="ps")
                        nc.tensor.matmul(ps[:], a1[0][:, bsl], th0[:], start=True, stop=False)
                        nc.tensor.matmul(ps[:], a1[1][:, bsl], th1[:], start=False, stop=False)
                    for t in tg:
                        bsl = slice(t * 128, (t + 1) * 128)
                        ps = pss[t]
                        nc.tensor.matmul(ps[:], x2[:, :, bsl], y2t[:], start=False,
                                         stop=False, perf_mode=DR)
                        nc.tensor.matmul(ps[:], x3[:, :, bsl], y3t[:], start=False,
                                         stop=True, perf_mode=DR)
                        sc = scr.tile([128, D], f32, tag="sc")
                        nc.vector.affine_mul_reduce(
                            out=sc[:], accum_out=bil[t][:, k:k + 1],
                            in0=ps[:, 0:D], in1=v2[t][:], scale=UNSCALE, bias=0.0,
                        )
                        sc2 = scr.tile([128, D], f32, tag="sc2")
                        nc.vector.affine_mul_reduce(
                            out=sc2[:], accum_out=bil[t][:, k + 1:k + 2],
                            in0=ps[:, D:2 * D], in1=v2[t][:], scale=UNSCALE, bias=0.0,
                        )
                        if kp == n_kp - 1:
                            # epilogue per tile as soon as its last AMR lands
                            pre = scr.tile([128, K], f32, name="pre", tag="pre")
                            nc.vector.scalar_tensor_tensor(
                                pre[:], ffp[:, t * K:(t + 1) * K], 2.0 ** -11,
                                bil[t][:],
                                mybir.AluOpType.mult, mybir.AluOpType.add,
                            )
                            ot = scr.tile([128, K], f32, name="ot", tag="ot")
                            nc.scalar.activation(
                                ot[:], pre[:], mybir.ActivationFunctionType.Tanh,
                            )
                            nc.sync.dma_start(
                                OUT.ap()[t * 128:(t + 1) * 128, :], ot[:])

    nc.compile()
    return nc


def _f8(x):
    return np.clip(x, -240.0, 240.0).astype(e4m3)


def _dr_lhs(x):
    """[BS, 256] -> [128, 2, BS] with d = ko*128 + ki."""
    return np.ascontiguousarray(x.T.reshape(2, 128, -1).transpose(1, 0, 2))


def _prep_inputs(V1, V2, T, W, b):
    V1 = np.asarray(V1, np.float32)
    V2 = np.asarray(V2, np.float32)
    T = np.asarray(T, np.float32)
    W = np.asarray(W, np.float32)
    b = np.asarray(b, np.float32)

    Ts = T * np.float32(S)
    TH = Ts.astype(np.float16)
    TL = Ts - TH.astype(np.float32)
    # DR layout [K, 128, 2, D]: (k, ki, ko, e) = val[k, ko*128+ki, e]
    Y2f = _f8(T * np.float32(SY)).reshape(K, 2, 128, D).transpose(0, 2, 1, 3)
    Y3f = _f8(TL * np.float32(SY)).reshape(K, 2, 128, D).transpose(0, 2, 1, 3)
    Y2f = np.ascontiguousarray(Y2f)
    Y3f = np.ascontiguousarray(Y3f)

    V1s = V1 * np.float32(S)
    A1f = V1s.astype(np.float16)
    A2 = V1s - A1f.astype(np.float32)
    X2f = _f8(A2 * np.float32(SX))
    X3f = _f8(A1f.astype(np.float32) * np.float32(SX / S))

    # ff with sum(b) folded in: CT gets a ones-row block, W a sum_b row.
    CTf = np.concatenate([V1, V2], axis=1)  # [B, 512]
    sum_b = np.float32(b.sum(dtype=np.float64))
    Wx = np.zeros((NC_FF * 128, K), dtype=np.float32)
    Wx[:512] = W
    Wx[512, :] = sum_b
    WHf = Wx.astype(np.float16)
    WLf = (Wx - WHf.astype(np.float32)).astype(np.float16)

    in_maps = []
    for c in range(NCORES):
        sl = slice(c * BS, (c + 1) * BS)
        CTx = np.zeros((NC_FF * 128, BS), dtype=np.float32)
        CTx[:512] = CTf[sl].T
        CTx[512, :] = 1.0
        CTx *= np.float32(S)
        CTHf = CTx.astype(np.float16)
        CTLf = (CTx - CTHf.astype(np.float32)).astype(np.float16)
        in_maps.append({
            "A1": np.ascontiguousarray(A1f[sl].T),
            "X2": _dr_lhs(X2f[sl]),
            "X3": _dr_lhs(X3f[sl]),
            "TH": TH,
            "Y2": Y2f,
            "Y3": Y3f,
            "V2N": V2[sl],
            "CTH": CTHf,
            "CTL": CTLf,
            "WH": WHf,
            "WL": WLf,
        })
    return in_maps


def kernel(V1, V2, T, W, b):
    if "nc" not in _NC_CACHE:
        _NC_CACHE["nc"] = _build()
    nc = _NC_CACHE["nc"]
    in_maps = _prep_inputs(V1, V2, T, W, b)
    res = bass_utils.run_bass_kernel_spmd(nc, in_maps, core_ids=list(range(NCORES)))
    return np.concatenate([r["OUT"] for r in res.results], axis=0)
